# revision 8
# baseline (speedup 1.0000x reference)
"""Dead-zone squared-error mean over N=33554432 elements, data-parallel on 8 NeuronCores.

reference:  diff = inputs - targets
            dz   = where(|diff| < 0.1, 0, diff)
            out  = mean(dz * dz)            (scalar float32)

Strategy: shard N across 8 cores (4,194,304 elements each).  Per core, stream
[128 x CHUNK] f32 tiles of both operands from HBM, compute
    d = x - t                 (DVE)
    s = d^2                   (ACT, Square)
    r = (s >= 0.01) * s       (DVE scalar_tensor_tensor, fused mask+mul)
with the per-partition running sum captured by the instruction's accum_out.
Each core returns a [128, NT] stats block (one column per tile); the host sums
the 8*128*NT partials in float64 and divides by N.
"""

import numpy as np

import concourse.bacc as bacc
import concourse.mybir as mybir
import concourse.tile as tile
from concourse.alu_op_type import AluOpType
from concourse.bass_utils import run_bass_kernel_spmd

N = 33554432
NCORES = 8
PER_CORE = N // NCORES          # 4194304
P = 128
CHUNK = 2048                    # free elems per tile -> 1 MiB per DMA
NT = PER_CORE // (P * CHUNK)    # 16 tiles per core
NSPLIT = 4                      # last tile is split into NSPLIT sub-tiles
TAILC = CHUNK // NSPLIT         # 512
NCOL = NT - 1 + NSPLIT          # stats columns
THRESH_SQ = 0.01                # (dead-zone 0.1)^2

F32 = mybir.dt.float32

_CACHE = {}


def _build_nc():
    nc = bacc.Bacc()
    x = nc.dram_tensor("x", [NT, P, CHUNK], F32, kind="ExternalInput")
    t = nc.dram_tensor("t", [NT, P, CHUNK], F32, kind="ExternalInput")
    out = nc.dram_tensor("out", [P, NCOL], F32, kind="ExternalOutput")

    with tile.TileContext(nc) as tc:
        with (
            tc.tile_pool(name="io", bufs=3) as io_pool,
            tc.tile_pool(name="tmp", bufs=2) as tmp_pool,
            tc.tile_pool(name="stats", bufs=1) as stats_pool,
        ):
            stats = stats_pool.tile([P, NCOL], F32)

            def process(x_ap, t_ap, c, col, tag):
                xt = io_pool.tile([P, c], F32, tag="x" + tag)
                tt = io_pool.tile([P, c], F32, tag="t" + tag)
                nc.sync.dma_start(out=xt[:], in_=x_ap)
                nc.sync.dma_start(out=tt[:], in_=t_ap)
                d = tmp_pool.tile([P, c], F32, tag="d" + tag)
                nc.vector.tensor_sub(d[:], xt[:], tt[:])
                s = tmp_pool.tile([P, c], F32, tag="s" + tag)
                nc.scalar.activation(s[:], d[:], mybir.ActivationFunctionType.Square)
                # r = (s >= 0.01) * s ; stats[:, col] = per-partition sum of r
                nc.vector.scalar_tensor_tensor(
                    out=d[:],
                    in0=s[:],
                    scalar=THRESH_SQ,
                    in1=s[:],
                    op0=AluOpType.is_ge,
                    op1=AluOpType.mult,
                    accum_out=stats[:, col : col + 1],
                )

            # bulk tiles at full CHUNK; last tile split into small sub-tiles
            # so the post-DMA serial chain (sub -> square -> mask+reduce) is
            # short when the final bytes land.
            for i in range(NT - 1):
                process(x[i], t[i], CHUNK, i, "b")
            for j in range(NSPLIT):
                lo, hi = j * TAILC, (j + 1) * TAILC
                process(
                    x[NT - 1][:, lo:hi], t[NT - 1][:, lo:hi], TAILC, NT - 1 + j, "s"
                )
            nc.sync.dma_start(out=out[:], in_=stats[:])
    nc.finalize()
    return nc


def kernel(inputs: np.ndarray, targets: np.ndarray) -> np.ndarray:
    x = np.ascontiguousarray(inputs, dtype=np.float32).reshape(NCORES, NT, P, CHUNK)
    t = np.ascontiguousarray(targets, dtype=np.float32).reshape(NCORES, NT, P, CHUNK)

    if "nc" not in _CACHE:
        _CACHE["nc"] = _build_nc()
    nc = _CACHE["nc"]

    in_maps = [{"x": x[c], "t": t[c]} for c in range(NCORES)]
    res = run_bass_kernel_spmd(nc, in_maps, list(range(NCORES)))

    total = 0.0
    for r in res.results:
        total += r["out"].astype(np.float64).sum()
    return np.array(total / N, dtype=np.float32)
